# revision 39
# baseline (speedup 1.0000x reference)
"""Multi-headed attention on 8 TRN2 NeuronCores (Bass/Tile).

Problem: x[4, 2048, 1024] f32; 16 heads, Dk=64.
  Q = x@Wq+bq, K = x@Wk+bk, V = x@Wv+bv  (per-head split)
  out = softmax(QK^T/8) V  re-merged, @Wo + bo

Sharding (tensor-parallel heads x batch): core = b*2 + hg
  b  in 0..3  : batch index
  hg in 0..1  : head group (8 heads = 512 of the 1024 d_model dims)
Each core gets x[b]^T (pre-transposed on host, bf16) and the hg-slice of the
weights, and produces the partial Y^T = (P V_hg) @ Wo_hg  (d-major, f32,
no biases). Host sums the two head-group partials per batch, transposes, and
adds bo + bv@Wo (the V-bias commutes through softmax: rows of P sum to 1).

On-core dataflow (all matmul operands bf16, PSUM f32):
  Xt   [1024,2048] d-major input (host-provided)
  Qt,Kt[512,2048]  d-major projections; bias added during PSUM->SBUF copy
  Vaug [2048, 8,65] natural V with a ones column per head (rowsum trick)
  per (q-block 512, head-pair): St^T [128k,2x512q] psum  (2 heads packed in
     the 128-row PE array via base-partition 0/64 row tiling, K=64 each)
  P~ = exp(St^T / 8) -> bf16 (one ACT op per [128,1024] tile; no max-sub:
     |scores| <~ 2 for this problem's distribution, exp is safe in f32)
  O^T+rowsum = [V_h | 1]^T @ P~^T  accumulated over 16 k-chunks -> [65, 512]
  Ot = O^T * (1/rowsum broadcast)  -> bf16  (odd heads DMA-shifted to
     partitions 64..127 so the final matmul sees full 128-row d-chunks)
  Y^T = Wo^T @ Ot  accumulated over 4 d-chunks -> f32 -> DRAM

Scheduling: the ACT exp stream (33.5M elems/core ~ 266us at 1 elem/lane/cyc)
is the bottleneck engine; everything is organized to keep it saturated:
  - lag-1 software pipeline at k-chunk granularity: unit u's score groups
    (drip-gated by the 2-buffer score PSUM ring at ACT pace) interleave
    with unit u-1's PV accumulation so the in-order PE never head-of-line
    blocks the next score group;
  - K/V projections for iteration n ride the lightly-loaded tail units of
    iteration n-1 (first iteration bootstraps them inside q-block 0);
  - output projections are split 4+4 groups into units (qb+1,1)/(qb+1,2),
    after the softmax-denominator DMA round trip has settled;
  - engine warmups (ACT table preload, PE HAM keep-alive) overlap the
    input DMAs; x and the j=0 weight slices are DMAed first so the first
    exp fires ~20us in.
"""

import os
import numpy as np
import ml_dtypes
from contextlib import ExitStack

import jax
from jax.sharding import Mesh, PartitionSpec
from jax.experimental.shard_map import shard_map

import concourse.bass as bass
import concourse.tile as tile
from concourse import bacc, mybir
from concourse import bass2jax

BF16 = ml_dtypes.bfloat16

B, S, D, H, DK = 4, 2048, 1024, 16, 64
HPG = 8              # heads per group (per core)
DS = HPG * DK        # 512: d_model slice per core
N_CORES = 8
P = 128
QW = 512             # q block width
QB = S // QW         # 4 q blocks
KC = D // P          # 8 contraction chunks for projections
DC = DS // P         # 4 d-chunks of the head-group slice (= head pairs)
TC = S // P          # 16 token chunks (= k_tok chunks)
FP32 = mybir.dt.float32
BF = mybir.dt.bfloat16
AF = mybir.ActivationFunctionType


# ablation switches for performance bisection (all True = full kernel)
ABLATE = {"exp": True, "pv": True, "norm": True, "final": True}


def build_tile_kernel(ctx: ExitStack, tc_ctx: tile.TileContext,
                      xt, wq, wk, wv, wo, bq, bk, yt, repeat=1):
    nc = tc_ctx.nc
    tc = tc_ctx

    wpool = ctx.enter_context(tc.tile_pool(name="w", bufs=1))
    xpool = ctx.enter_context(tc.tile_pool(name="x", bufs=1))
    qkpool = ctx.enter_context(tc.tile_pool(name="qk", bufs=1))
    vpool = ctx.enter_context(tc.tile_pool(name="v", bufs=1))
    opool = ctx.enter_context(tc.tile_pool(name="o", bufs=1))
    ptpool = ctx.enter_context(tc.tile_pool(name="pt", bufs=24))
    small = ctx.enter_context(tc.tile_pool(name="small", bufs=3))
    ypool = ctx.enter_context(tc.tile_pool(name="y", bufs=3))
    psA = ctx.enter_context(tc.tile_pool(name="psA", bufs=2, space="PSUM"))
    psB = ctx.enter_context(tc.tile_pool(name="psB", bufs=2, space="PSUM"))
    psOp = ctx.enter_context(tc.tile_pool(name="psO", bufs=2, space="PSUM"))
    dscr = ctx.enter_context(tc.tile_pool(name="dscr", bufs=4, space="DRAM"))

    # ---- engine warmups, before any DMA-gated work ----
    # ACT: a dummy exp forces the ~2.7us LoadActFuncSet to overlap the
    # input DMAs instead of delaying the first real exp.
    jk = wpool.tile([P, 8], FP32)
    nc.vector.memset(jk[:], 0.0)
    nc.scalar.activation(jk[:], jk[:], AF.Exp)
    # PE: junk matmuls keep the HAM activity window busy during the input
    # DMA wait so the first real matmuls run at 2.4 GHz, not 1.2.
    jw = wpool.tile([P, QW], BF)
    nc.vector.memset(jw[:], 0.0)
    jps = psB.tile([P, QW], FP32, tag="b")
    for i in range(12):
        nc.tensor.matmul(jps[0:64, :], lhsT=jw[:, 0:64], rhs=jw[:],
                         start=(i == 0), stop=(i == 11))

    # ---- inputs -> SBUF. x is DMAed in (kc, tb) chunks so the first
    # K-projection chunk can start after ~1MB instead of 4MB; everything
    # else in rough order of first use. ----
    w_q = wpool.tile([P, KC, DS], BF)
    w_k = wpool.tile([P, KC, DS], BF)
    w_v = wpool.tile([P, KC, DS], BF)
    x_sb = xpool.tile([P, KC, S], BF)
    bq_sb = wpool.tile([P, DC], FP32)
    bk_sb = wpool.tile([P, DC], FP32)
    nc.sync.dma_start(bq_sb[:], bq.rearrange("(c p) -> p c", p=P))
    nc.sync.dma_start(bk_sb[:], bk.rearrange("(c p) -> p c", p=P))
    # first-scores gate: x token-block 0 + the j=0 column slices of Wq/Wk,
    # interleaved per contraction chunk so the first projections pipeline
    # with DMA arrival
    for kc in range(KC):
        nc.sync.dma_start(x_sb[:, kc, 0:QW], xt[kc * P:(kc + 1) * P, 0:QW])
        nc.sync.dma_start(w_q[:, kc, 0:P], wq[kc * P:(kc + 1) * P, 0:P])
        nc.sync.dma_start(w_k[:, kc, 0:P], wk[kc * P:(kc + 1) * P, 0:P])
    for kc in range(KC):
        nc.sync.dma_start(w_v[:, kc], wv[kc * P:(kc + 1) * P, :])
    for tb in range(1, QB):
        for kc in range(KC):
            nc.sync.dma_start(x_sb[:, kc, tb * QW:(tb + 1) * QW],
                              xt[kc * P:(kc + 1) * P, tb * QW:(tb + 1) * QW])
    for kc in range(KC):
        nc.sync.dma_start(w_q[:, kc, P:DS], wq[kc * P:(kc + 1) * P, P:DS])
        nc.sync.dma_start(w_k[:, kc, P:DS], wk[kc * P:(kc + 1) * P, P:DS])
    w_o = wpool.tile([P, DC, D], BF)
    for dc in range(DC):
        nc.sync.dma_start(w_o[:, dc], wo[dc * P:(dc + 1) * P, :])

    qt = qkpool.tile([P, DC, S], BF)
    kt = qkpool.tile([P, DC, S], BF)
    vaug = vpool.tile([P, TC, HPG, DK + 1], BF)
    ot = opool.tile([P, DC, S], BF)

    nc.vector.memset(vaug[:, :, :, DK], 1.0)  # ones column for rowsums

    def qk_proj(w_sb, b_sb, dest, j, tb):
        """Project one [128 dims x 512 tokens] block of Q^T or K^T."""
        ps = psB.tile([P, QW], FP32, tag="b")
        for kc in range(KC):
            nc.tensor.matmul(
                ps[:],
                lhsT=w_sb[:, kc, j * P:(j + 1) * P],
                rhs=x_sb[:, kc, tb * QW:(tb + 1) * QW],
                start=(kc == 0), stop=(kc == KC - 1))
        nc.vector.tensor_scalar_add(
            dest[:, j, tb * QW:(tb + 1) * QW], ps[:], b_sb[:, j:j + 1])

    def v_proj_chunk(tci):
        ps = psB.tile([P, DS], FP32, tag="b")
        for kc in range(KC):
            nc.tensor.matmul(
                ps[:],
                lhsT=x_sb[:, kc, tci * P:(tci + 1) * P],
                rhs=w_v[:, kc, :],
                start=(kc == 0), stop=(kc == KC - 1))
        nc.vector.tensor_copy(
            vaug[:, tci, :, 0:DK], ps.rearrange("p (h e) -> p h e", e=DK))

    # ---- lag-1 software pipeline over the 16 (qb, j) attention units ----
    # Each unit emits its scores+exp stream interleaved (per k-chunk) with
    # the PREVIOUS unit's PV accumulation, so the PE always has ready work
    # between ACT-gated score groups and the exp stream never stalls at
    # unit/qb/iteration boundaries. Output projections for q-block qb are
    # emitted inside unit (qb+1, 0) for the same reason.

    class Unit:
        def __init__(self, qb, j):
            self.qb, self.j = qb, j
            self.pts = [None] * TC
            self.psOs = None

    def scores_exp_step(u, kc2):
        psS = psA.tile([P, 2 * QW], FP32, tag="s")
        for h01 in range(2):
            lo = h01 * DK
            nc.tensor.matmul(
                psS[:, h01 * QW:(h01 + 1) * QW],
                lhsT=kt[lo:lo + DK, u.j, kc2 * P:(kc2 + 1) * P],
                rhs=qt[lo:lo + DK, u.j, u.qb * QW:(u.qb + 1) * QW],
                start=True, stop=True)
        if ABLATE["exp"]:
            pt = ptpool.tile([P, 2 * QW], BF, tag="pt")
            nc.scalar.activation(pt[:], psS[:], AF.Exp, scale=0.125)
            u.pts[kc2] = pt

    def pv_step(u, kc2):
        if not (ABLATE["exp"] and ABLATE["pv"]):
            return
        if u.psOs is None:
            u.psOs = [psOp.tile([P, QW], FP32, tag="psO", name=f"psO{_h}")
                      for _h in range(2)]
        for h01 in range(2):
            nc.tensor.matmul(
                u.psOs[h01][0:DK + 1, :],
                lhsT=vaug[:, kc2, 2 * u.j + h01, :],
                rhs=u.pts[kc2][:, h01 * QW:(h01 + 1) * QW],
                start=(kc2 == 0), stop=(kc2 == TC - 1))

    def normalize(u):
        if not (ABLATE["exp"] and ABLATE["pv"]):
            return
        qb, j = u.qb, u.j
        for h01 in range(2):
            psO = u.psOs[h01]
            if not ABLATE["norm"]:
                nc.vector.tensor_copy(
                    ot[0:DK, j, qb * QW:(qb + 1) * QW], psO[0:DK, :])
                continue
            # one quick copy of O+rowsum to SBUF releases the PSUM slot
            # (~0.7us) instead of holding it through the multi-hop DMA
            # normalization chain (~4us)
            ou = small.tile([P, QW], FP32, tag="ou")
            nc.vector.tensor_copy(ou[0:DK + 1, :], psO[0:DK + 1, :])
            # 1/rowsum, computed on 64 partitions instead of one: DVE
            # reciprocal is ~6ns/elem *per lane*, so a [1,512] row costs
            # 3.1us while [64,8] costs ~0.2us. Rowsum row -> DRAM ->
            # reload as [64,8] -> reciprocal -> DRAM -> broadcast [64,512].
            # (SBUF APs cannot repartition or broadcast; DRAM hops can.
            # reciprocal_approx_fast / gpsimd.partition_broadcast are
            # broken on this runtime — standard ops only.)
            rd = dscr.tile([1, QW], FP32, tag="rd")
            nc.sync.dma_start(rd[:], ou[DK:DK + 1, :])
            rec = small.tile([P, 8], FP32, tag="rec")
            nc.sync.dma_start(rec[0:DK, :],
                              rd.rearrange("o (a b) -> (o a) b", b=8))
            nc.vector.reciprocal(rec[0:DK, :], rec[0:DK, :])
            rd2 = dscr.tile([1, QW], FP32, tag="rd2")
            nc.sync.dma_start(rd2.rearrange("o (a b) -> (o a) b", b=8),
                              rec[0:DK, :])
            bc = small.tile([P, QW], FP32, tag="bc")
            nc.sync.dma_start(bc[0:DK, :], rd2.to_broadcast((DK, QW)))
            if h01 == 0:
                nc.vector.tensor_mul(
                    ot[0:DK, j, qb * QW:(qb + 1) * QW],
                    ou[0:DK, :], bc[0:DK, :])
            else:
                tmp = small.tile([P, QW], BF, tag="tmp")
                nc.vector.tensor_mul(tmp[0:DK, :], ou[0:DK, :], bc[0:DK, :])
                nc.sync.dma_start(
                    ot[DK:P, j, qb * QW:(qb + 1) * QW], tmp[0:DK, :])

    def final_group(qb, oc):
        ps = psB.tile([P, QW], FP32, tag="b")
        for dc in range(DC):
            nc.tensor.matmul(
                ps[:],
                lhsT=w_o[:, dc, oc * P:(oc + 1) * P],
                rhs=ot[:, dc, qb * QW:(qb + 1) * QW],
                start=(dc == 0), stop=(dc == DC - 1))
        y_sb = ypool.tile([P, QW], FP32, tag="y")
        # explicit DVE: finals run alongside exps, and nc.any would
        # put these copies on the exp-critical ACT engine
        nc.vector.tensor_copy(y_sb[:], ps[:])
        nc.sync.dma_start(
            yt[oc * P:(oc + 1) * P, qb * QW:(qb + 1) * QW], y_sb[:])

    units = [(qb_, j_) for qb_ in range(QB) for j_ in range(DC)] * repeat
    n_units = len(units)
    prev = None
    for idx, (qb, j) in enumerate(units):
        iter_unit = idx % (QB * DC)
        bootstrap = idx < (QB * DC)
        u = Unit(qb, j)
        qk_proj(w_q, bq_sb, qt, j, qb)
        # Per-index extra PE work, spread so no unit's stream is much
        # heavier than one ACT period (~16.6us). All K/V projections for
        # steady-state iterations ride the lightly-loaded tail units of the
        # PREVIOUS iteration (values are identical across iterations, so
        # overwriting early is benign and Tile's dependency order keeps
        # execution consistent); the first iteration bootstraps them
        # in-place inside q-block 0. Output projections split 4+4 across
        # the two units after their q-block's last normalize.
        extras = {}
        pre_extras = {}

        def put(kc2, fn, *a):
            extras.setdefault(kc2, []).append((fn, a))

        def put_pre(kc2, fn, *a):
            pre_extras.setdefault(kc2, []).append((fn, a))
        if bootstrap:
            if qb == 0:
                # this unit's own scores consume these: emit BEFORE the
                # score group at the same index
                for tb_ in range(QB):
                    put_pre(4 * tb_, qk_proj, w_k, bk_sb, kt, j, tb_)
            if iter_unit < 2:
                for c in range(8):
                    put(c, v_proj_chunk, 8 * iter_unit + c)
        if not bootstrap:
            # this iteration's remaining self-prep (emitted ahead of use)
            if iter_unit == 0:
                put(0, qk_proj, w_k, bk_sb, kt, 3, 0)
                put(4, qk_proj, w_k, bk_sb, kt, 3, 1)
                for c in (12, 13):
                    put(2 * (c - 12) + 1, v_proj_chunk, c)
            if iter_unit == 1:
                for c in (14, 15):
                    put(2 * (c - 14), v_proj_chunk, c)
        if idx + QB * DC < n_units:
            # prep for the NEXT iteration in this one's tail units
            if iter_unit == 10:
                for c in (0, 1):
                    put(8 * c, v_proj_chunk, c)
            if iter_unit == 11:
                for c in (2, 3):
                    put(8 * (c - 2), v_proj_chunk, c)
            if iter_unit == 12:
                for c in (4, 5):
                    put(8 * (c - 4), v_proj_chunk, c)
            if iter_unit == 13:
                put(0, qk_proj, w_k, bk_sb, kt, 0, 0)
                put(4, qk_proj, w_k, bk_sb, kt, 0, 1)
                put(8, qk_proj, w_k, bk_sb, kt, 0, 2)
                put(12, qk_proj, w_k, bk_sb, kt, 0, 3)
                for c in (6, 7):
                    put(2 * (c - 6) + 2, v_proj_chunk, c)
            if iter_unit == 14:
                for tb_ in range(QB):
                    put(4 * tb_, qk_proj, w_k, bk_sb, kt, 1, tb_)
                for c in (8, 9):
                    put(2 * (c - 8) + 2, v_proj_chunk, c)
            if iter_unit == 15:
                for tb_ in range(QB):
                    put(4 * tb_, qk_proj, w_k, bk_sb, kt, 2, tb_)
                for c in (10, 11):
                    put(2 * (c - 10) + 2, v_proj_chunk, c)
                put(14, qk_proj, w_k, bk_sb, kt, 3, 2)
                put(15, qk_proj, w_k, bk_sb, kt, 3, 3)
        # output projection of q-block qb-2 (normalized two units ago),
        # 4 groups here in unit (qb, 1) and 4 in (qb, 2)
        if ABLATE["final"] and prev is not None:
            fqb = (qb + QB - 1) % QB
            if j == 1 and (idx >= 5):
                for g in range(4):
                    put(12 + g, final_group, fqb, g)
            if j == 2 and (idx >= 6):
                for g in range(4):
                    put(12 + g, final_group, fqb, g + 4)
        for kc2 in range(TC):
            for fn, a in pre_extras.get(kc2, ()):
                fn(*a)
            scores_exp_step(u, kc2)
            if prev is not None:
                pv_step(prev, kc2)
            for fn, a in extras.get(kc2, ()):
                fn(*a)
        if prev is not None:
            normalize(prev)
        prev = u
    # pipeline flush
    for kc2 in range(TC):
        pv_step(prev, kc2)
    normalize(prev)
    if ABLATE["final"]:
        # warm-keepers: the PE would otherwise idle through the ~4us
        # normalize round trip and run the last output projection at the
        # throttled clock
        for i in range(8):
            nc.tensor.matmul(jps[0:64, :], lhsT=jw[:, 0:64], rhs=jw[:],
                             start=(i == 0), stop=(i == 7))
        for oc in range(D // P):
            final_group(prev.qb, oc)


def build_module(repeat=1):
    nc = bacc.Bacc("TRN2", target_bir_lowering=False, debug=False)
    xt = nc.dram_tensor("xt", [D, S], BF, kind="ExternalInput").ap()
    wq = nc.dram_tensor("wq", [D, DS], BF, kind="ExternalInput").ap()
    wk = nc.dram_tensor("wk", [D, DS], BF, kind="ExternalInput").ap()
    wv = nc.dram_tensor("wv", [D, DS], BF, kind="ExternalInput").ap()
    wo = nc.dram_tensor("wo", [DS, D], BF, kind="ExternalInput").ap()
    bq = nc.dram_tensor("bq", [DS], FP32, kind="ExternalInput").ap()
    bk = nc.dram_tensor("bk", [DS], FP32, kind="ExternalInput").ap()
    yt = nc.dram_tensor("yt", [D, S], FP32, kind="ExternalOutput").ap()
    with tile.TileContext(nc) as tc:
        with ExitStack() as ctx:
            build_tile_kernel(ctx, tc, xt, wq, wk, wv, wo, bq, bk, yt,
                              repeat=repeat)
    nc.compile()
    return nc


def _collect_io(nc):
    partition_name = (nc.partition_id_tensor.name
                      if nc.partition_id_tensor else None)
    in_names, out_names, out_avals = [], [], []
    for alloc in nc.m.functions[0].allocations:
        if not isinstance(alloc, mybir.MemoryLocationSet):
            continue
        name = alloc.memorylocations[0].name
        if alloc.kind == "ExternalInput":
            if name != partition_name:
                in_names.append(name)
        elif alloc.kind == "ExternalOutput":
            out_names.append(name)
            out_avals.append(jax.core.ShapedArray(
                tuple(alloc.tensor_shape), mybir.dt.np(alloc.dtype)))
    return in_names, out_names, out_avals, partition_name


def make_runner(nc, donate=False):
    """Multi-core PJRT runner (the run_bass_via_pjrt path, but with the
    jitted executable retained so repeated calls don't re-lower).

    donate=False: the kernel writes every element of its outputs, so the
    zero output-operands never need to be donated; keeping them allows the
    same device-resident args to be re-used for repeated timed calls."""
    bass2jax.install_neuronx_cc_hook()
    in_names, out_names, out_avals, partition_name = _collect_io(nc)
    n_params, n_outs = len(in_names), len(out_names)
    all_names = in_names + out_names
    if partition_name is not None:
        all_names = all_names + [partition_name]

    def _body(*args):
        operands = list(args)
        if partition_name is not None:
            operands.append(bass2jax.partition_id_tensor())
        outs = bass2jax._bass_exec_p.bind(
            *operands,
            out_avals=tuple(out_avals),
            in_names=tuple(all_names),
            out_names=tuple(out_names),
            lowering_input_output_aliases=(),
            sim_require_finite=True,
            sim_require_nnan=True,
            nc=nc,
        )
        return tuple(outs)

    devices = jax.devices()[:N_CORES]
    mesh = Mesh(np.asarray(devices), ("core",))
    jit_kwargs = dict(keep_unused=True)
    if donate:
        jit_kwargs["donate_argnums"] = tuple(range(n_params, n_params + n_outs))
    sharded = jax.jit(
        shard_map(_body, mesh=mesh,
                  in_specs=(PartitionSpec("core"),) * (n_params + n_outs),
                  out_specs=(PartitionSpec("core"),) * n_outs,
                  check_rep=False),
        **jit_kwargs)

    def host_args(in_maps):
        concat_in = [
            np.concatenate([np.asarray(m[name]) for m in in_maps], axis=0)
            for name in in_names]
        concat_zeros = [
            np.zeros((N_CORES * a.shape[0],) + tuple(a.shape[1:]), a.dtype)
            for a in out_avals]
        return concat_in + concat_zeros

    def device_args(in_maps):
        from jax.sharding import NamedSharding
        args = host_args(in_maps)
        return [
            jax.device_put(a, NamedSharding(
                mesh, PartitionSpec("core", *(None,) * (a.ndim - 1))))
            for a in args]

    def run(in_maps, args=None):
        if args is None:
            args = host_args(in_maps)
        out_arrs = sharded(*args)
        return [
            {name: np.asarray(out_arrs[i]).reshape(
                (N_CORES,) + tuple(out_avals[i].shape))[c]
             for i, name in enumerate(out_names)}
            for c in range(N_CORES)]

    run.in_names = in_names
    run.out_names = out_names
    run.out_avals = out_avals
    run.sharded = sharded
    run.mesh = mesh
    run.host_args = host_args
    run.device_args = device_args
    return run


def shard_inputs(inputs):
    """Full problem inputs -> 8 per-core input maps (host-side prep)."""
    x = np.asarray(inputs["x"], dtype=np.float32)
    Wq = np.asarray(inputs["Wq"], dtype=np.float32)
    Wk = np.asarray(inputs["Wk"], dtype=np.float32)
    Wv = np.asarray(inputs["Wv"], dtype=np.float32)
    Wo = np.asarray(inputs["Wo"], dtype=np.float32)
    bq = np.asarray(inputs["bq"], dtype=np.float32)
    bk = np.asarray(inputs["bk"], dtype=np.float32)
    in_maps = []
    for b in range(B):
        xt_b = np.ascontiguousarray(x[b].T).astype(BF16)
        for hg in range(2):
            sl = slice(hg * DS, (hg + 1) * DS)
            in_maps.append({
                "xt": xt_b,
                "wq": np.ascontiguousarray(Wq[:, sl]).astype(BF16),
                "wk": np.ascontiguousarray(Wk[:, sl]).astype(BF16),
                "wv": np.ascontiguousarray(Wv[:, sl]).astype(BF16),
                "wo": np.ascontiguousarray(Wo[sl, :]).astype(BF16),
                "bq": np.ascontiguousarray(bq[sl]),
                "bk": np.ascontiguousarray(bk[sl]),
            })
    return in_maps


def gather_output(results, inputs):
    Wo = np.asarray(inputs["Wo"], dtype=np.float32)
    bv = np.asarray(inputs["bv"], dtype=np.float32)
    bo = np.asarray(inputs["bo"], dtype=np.float32)
    bias = bo + bv @ Wo  # V-bias passes through softmax (rows of P sum to 1)
    out = np.empty((B, S, D), dtype=np.float32)
    for b in range(B):
        acc = results[2 * b]["yt"] + results[2 * b + 1]["yt"]  # [D, S]
        out[b] = acc.T + bias
    return out


_CACHE = {}


def _get_runner():
    if "runner" not in _CACHE:
        nc = build_module()
        _CACHE["nc"] = nc
        _CACHE["runner"] = make_runner(nc)
    return _CACHE["runner"]


def kernel(**inputs) -> np.ndarray:
    runner = _get_runner()
    in_maps = shard_inputs(inputs)
    results = runner(in_maps)
    return gather_output(results, inputs)

